# revision 6
# baseline (speedup 1.0000x reference)
"""CARC attention processor kernel for 8 Trainium2 NeuronCores.

Reference computation (B=1, L=4096, C=640, H=10, D=64):
    q/k/v = hidden @ Wq/Wk/Wv, split into 10 heads of 64
    k_cat = [k, 0.42*K_bg], v_cat = [v, 0.42*V_bg]   (key length 8192)
    out   = softmax(q k_cat^T / 8) v_cat, heads merged, @ Wo + bo

Sharding: queries are split 512 per core; every core computes all 10 heads
for its queries (k/v projections replicated per core — cheap relative to
attention).  Output is a disjoint row-slice per core; host just concatenates.

All matmuls run in bf16 with fp32 PSUM accumulation.  Softmax skips the
max-subtraction (scores are ~N(0,1); exp is computed on the ScalarE with the
1/8 scale folded in, and the 0.42 key-side scale folded into the bg exp
scale).  The softmax denominator comes from a ones-column appended to V in
the probs@V matmul; the output-projection bias is folded in as a 65th row of
Wo against the ctx ones-column.
"""

import numpy as np

import concourse.bass as bass
import concourse.mybir as mybir
import concourse.tile as tile
from concourse.masks import make_identity

F32 = mybir.dt.float32
BF16 = mybir.dt.bfloat16
AF = mybir.ActivationFunctionType

# Problem constants (hardcoded per contract)
B, L, C = 1, 4096, 640
H, D = 10, 64
ALPHA = 0.42
N_CORES = 8
SCALE = 1.0 / np.sqrt(D)  # 0.125


class Cfg:
    def __init__(self, H=H, C=C, Lk=L, Q=L // N_CORES):
        assert C % 128 == 0 and Lk % 512 == 0 and Q % 128 == 0 and Q <= 512
        self.H, self.C, self.Lk, self.Q = H, C, Lk, Q
        self.n_cc = C // 128      # contraction chunks for projections
        self.n_kt = Lk // 128     # key tiles per source (self / bg)
        self.n_kc = Lk // 512     # 512-wide column groups of keys
        self.n_qt = Q // 128      # query tiles of this core


def emit(nc: bass.Bass, cfg: Cfg):
    Hh, Cc, Lk, Q = cfg.H, cfg.C, cfg.Lk, cfg.Q
    n_cc, n_kt, n_qt = cfg.n_cc, cfg.n_kt, cfg.n_qt

    hT = nc.declare_dram_parameter("hT", [Cc, Lk], F32, isOutput=False)
    hqT = nc.declare_dram_parameter("hqT", [Cc, Q], F32, isOutput=False)
    kbgT = nc.declare_dram_parameter("KbgT", [Hh, D, Lk], F32, isOutput=False)
    vbg = nc.declare_dram_parameter("Vbg", [Hh, Lk, D], F32, isOutput=False)
    wq = nc.declare_dram_parameter("Wq", [Cc, Cc], F32, isOutput=False)
    wk = nc.declare_dram_parameter("Wk", [Cc, Cc], F32, isOutput=False)
    wv = nc.declare_dram_parameter("Wv", [Cc, Cc], F32, isOutput=False)
    wob = nc.declare_dram_parameter("WoB", [Hh, D + 1, Cc], F32, isOutput=False)
    out = nc.declare_dram_parameter("out", [Q, Cc], F32, isOutput=True)

    with tile.TileContext(nc) as tc:
        with (
            tc.tile_pool(name="singles", bufs=1) as singles,
            tc.tile_pool(name="stage", bufs=1) as stage,
            tc.tile_pool(name="bgstage", bufs=2) as bgstage,
            tc.tile_pool(name="kv", bufs=2) as kv,
            tc.tile_pool(name="probs", bufs=4) as probs_pool,
            tc.tile_pool(name="outsb", bufs=2) as outsb_pool,
            tc.tile_pool(name="ps_a", bufs=2, space="PSUM") as ps_a,
            tc.tile_pool(name="ps_sc", bufs=2, space="PSUM") as ps_sc,
            tc.tile_pool(name="ps_ctx", bufs=1, space="PSUM") as ps_ctx,
        ):
            # ---- persistent SBUF tensors ----
            hT_bf = singles.tile([128, n_cc, Lk], BF16, tag="hT_bf")
            hq_bf = singles.tile([128, n_cc, Q], BF16, tag="hq_bf")
            wq_bf = singles.tile([128, n_cc, Cc], BF16, tag="wq_bf")
            wk_bf = singles.tile([128, n_cc, Cc], BF16, tag="wk_bf")
            wv_bf = singles.tile([128, n_cc, Cc], BF16, tag="wv_bf")
            wob_bf = singles.tile([D + 1, Hh, Cc], BF16, tag="wob_bf")
            qT_all = singles.tile([D, Hh, Q], BF16, tag="qT_all")
            ctx_all = singles.tile([128, Hh * n_qt, D + 1], BF16, tag="ctx_all")
            ctxT_all = singles.tile([D + 1, Hh * n_qt, 128], BF16, tag="ctxT_all")
            ident = singles.tile([128, 128], BF16, tag="ident")
            make_identity(nc, ident)

            # ---- load + cast hidden (transposed) and weights ----
            for i in range(n_cc):
                st = stage.tile([128, Lk], F32, tag="stage")
                nc.sync.dma_start(out=st, in_=hT[128 * i : 128 * (i + 1), :])
                nc.vector.tensor_copy(out=hT_bf[:, i, :], in_=st)
            for i in range(n_cc):
                st = stage.tile([128, Q], F32, tag="stage")
                nc.sync.dma_start(out=st, in_=hqT[128 * i : 128 * (i + 1), :])
                nc.vector.tensor_copy(out=hq_bf[:, i, :], in_=st)
            for w_dram, w_sb in ((wq, wq_bf), (wk, wk_bf), (wv, wv_bf)):
                st = stage.tile([128, n_cc, Cc], F32, tag="stage")
                nc.sync.dma_start(
                    out=st, in_=w_dram.rearrange("(i p) n -> p i n", p=128)
                )
                nc.vector.tensor_copy(out=w_sb, in_=st)
            hh = Hh // 2
            for half in range(2):
                st = stage.tile([D + 1, hh, Cc], F32, tag="stage")
                nc.sync.dma_start(
                    out=st,
                    in_=wob[half * hh : (half + 1) * hh].rearrange("h p n -> p h n"),
                )
                nc.vector.tensor_copy(
                    out=wob_bf[:, half * hh : (half + 1) * hh, :], in_=st
                )

            # ---- q projections: qT_all[:, h, :] = (hq @ Wq_h)^T ----
            for h in range(Hh):
                ps = ps_a.tile([D, Q], F32, tag="ps_a")
                for i in range(n_cc):
                    nc.tensor.matmul(
                        ps,
                        lhsT=wq_bf[:, i, D * h : D * (h + 1)],
                        rhs=hq_bf[:, i, :],
                        start=(i == 0),
                        stop=(i == n_cc - 1),
                    )
                nc.vector.tensor_copy(out=qT_all[:, h, :], in_=ps)

            # ---- per-head: project k/v, load bg kv, attention ----
            for h in range(Hh):
                kT_h = kv.tile([D, Lk], BF16, tag="kT")
                v_h = kv.tile([128, n_kt, D + 1], BF16, tag="v")
                kbg_h = kv.tile([D, Lk], BF16, tag="kbg")
                vbg_h = kv.tile([128, n_kt, D + 1], BF16, tag="vbg")

                # kT_h = (hidden @ Wk_h)^T  as [D, Lk]
                for t in range(Lk // 512):
                    ps = ps_a.tile([D, 512], F32, tag="ps_a")
                    for i in range(n_cc):
                        nc.tensor.matmul(
                            ps,
                            lhsT=wk_bf[:, i, D * h : D * (h + 1)],
                            rhs=hT_bf[:, i, 512 * t : 512 * (t + 1)],
                            start=(i == 0),
                            stop=(i == n_cc - 1),
                        )
                    nc.vector.tensor_copy(
                        out=kT_h[:, 512 * t : 512 * (t + 1)], in_=ps
                    )
                # v_h natural [keys, D] (+ones col)
                for kt in range(n_kt):
                    ps = ps_a.tile([128, D], F32, tag="ps_a")
                    for i in range(n_cc):
                        nc.tensor.matmul(
                            ps,
                            lhsT=hT_bf[:, i, 128 * kt : 128 * (kt + 1)],
                            rhs=wv_bf[:, i, D * h : D * (h + 1)],
                            start=(i == 0),
                            stop=(i == n_cc - 1),
                        )
                    nc.vector.tensor_copy(out=v_h[:, kt, 0:D], in_=ps)
                nc.vector.memset(v_h[:, :, D : D + 1], 1.0)

                # bg K (transposed) and bg V (scaled by ALPHA at load),
                # staged in 1/4 pieces to bound SBUF staging space
                for p4 in range(4):
                    lw = Lk // 4
                    st = bgstage.tile([D, lw], F32, tag="kbg_st", name=f"kst{h}{p4}")
                    nc.sync.dma_start(
                        out=st, in_=kbgT[h, :, lw * p4 : lw * (p4 + 1)]
                    )
                    nc.vector.tensor_copy(
                        out=kbg_h[:, lw * p4 : lw * (p4 + 1)], in_=st
                    )
                    tw = n_kt // 4
                    st2 = bgstage.tile(
                        [128, tw, D], F32, tag="vbg_st", name=f"vst{h}{p4}"
                    )
                    nc.sync.dma_start(
                        out=st2,
                        in_=vbg[h, lw * p4 : lw * (p4 + 1), :].rearrange(
                            "(kt p) d -> p kt d", p=128
                        ),
                    )
                    nc.vector.tensor_scalar_mul(
                        vbg_h[:, tw * p4 : tw * (p4 + 1), 0:D], st2, ALPHA
                    )
                nc.vector.memset(vbg_h[:, :, D : D + 1], 1.0)

                # attention for this head
                ctx_ps = [
                    ps_ctx.tile(
                        [128, D + 1], F32, tag=f"ctx{qt}", name=f"ctx_ps_{h}_{qt}"
                    )
                    for qt in range(n_qt)
                ]
                for src in range(2):  # 0=self keys, 1=bg keys
                    kT_src = kT_h if src == 0 else kbg_h
                    v_src = v_h if src == 0 else vbg_h
                    e_scale = SCALE if src == 0 else SCALE * ALPHA
                    for kt in range(n_kt):
                        sc = ps_sc.tile([128, Q], F32, tag="sc")
                        nc.tensor.matmul(
                            sc,
                            lhsT=kT_src[:, 128 * kt : 128 * (kt + 1)],
                            rhs=qT_all[:, h, :],
                            start=True,
                            stop=True,
                        )
                        pr = probs_pool.tile([128, Q], BF16, tag="pr")
                        nc.scalar.activation(pr, sc, AF.Exp, scale=e_scale)
                        first = src == 0 and kt == 0
                        last = src == 1 and kt == n_kt - 1
                        for qt in range(n_qt):
                            nc.tensor.matmul(
                                ctx_ps[qt],
                                lhsT=pr[:, 128 * qt : 128 * (qt + 1)],
                                rhs=v_src[:, kt, :],
                                start=first,
                                stop=last,
                            )
                # normalize + transpose ctx for the output projection
                for qt in range(n_qt):
                    j = h * n_qt + qt
                    r = probs_pool.tile([128, 1], F32, tag="recip")
                    nc.vector.reciprocal(r, ctx_ps[qt][:, D : D + 1])
                    nc.vector.tensor_scalar_mul(
                        ctx_all[:, j, 0:D], ctx_ps[qt][:, 0:D], r
                    )
                    nc.vector.memset(ctx_all[:, j, D : D + 1], 1.0)
                    tr = ps_a.tile([D + 1, 128], BF16, tag="ps_a")
                    nc.tensor.transpose(tr, ctx_all[:, j, :], ident)
                    nc.vector.tensor_copy(out=ctxT_all[:, j, :], in_=tr)

            # ---- output projection: out[qt] = sum_h ctxT_h^T @ WoB_h ----
            for qt in range(n_qt):
                o_sb = outsb_pool.tile([128, Cc], F32, tag="o_sb")
                for n0 in range(0, Cc, 512):
                    nw = min(512, Cc - n0)
                    ps = ps_sc.tile([128, nw], F32, tag="sc")
                    for h in range(Hh):
                        nc.tensor.matmul(
                            ps,
                            lhsT=ctxT_all[:, h * n_qt + qt, :],
                            rhs=wob_bf[:, h, n0 : n0 + nw],
                            start=(h == 0),
                            stop=(h == Hh - 1),
                        )
                    nc.vector.tensor_copy(out=o_sb[:, n0 : n0 + nw], in_=ps)
                nc.sync.dma_start(
                    out=out[128 * qt : 128 * (qt + 1), :], in_=o_sb
                )
    return nc


def split_waits(nc, limit=1):
    """This container's walrus rejects >~2 sync waits per instruction; hoist
    excess waits onto standalone EventSemaphore instructions placed before."""
    cnt = 0
    for f in nc.m.functions:
        for bb in f.blocks:
            fixed = []
            for inst in bb.instructions:
                si = inst.sync_info
                if si is not None and len(si.on_wait) > limit:
                    waits = list(si.on_wait)
                    extra, keep = waits[:-limit], waits[-limit:]
                    for w in extra:
                        cnt += 1
                        ev = mybir.InstEventSemaphore(
                            name=f"I-waitsplit-{cnt}", ins=[], outs=[]
                        )
                        ev.engine = inst.engine
                        ev.sync_info = mybir.SyncInfo(on_wait=[w], on_update=[])
                        nc.register_instruction(ev)
                        fixed.append(ev)
                    si.on_wait = keep
                fixed.append(inst)
            bb.instructions[:] = fixed
    return cnt


def build_bass(cfg: Cfg | None = None):
    cfg = cfg or Cfg()
    nc = bass.Bass()
    emit(nc, cfg)
    split_waits(nc)
    return nc


def make_in_maps(hidden_states, K_bg, V_bg, Wq, Wk, Wv, Wo, bo):
    hT = np.ascontiguousarray(np.asarray(hidden_states, np.float32)[0].T)
    KbgT = np.ascontiguousarray(np.asarray(K_bg, np.float32).transpose(0, 2, 1))
    WoB = np.zeros((H, D + 1, C), np.float32)
    WoB[:, :D, :] = np.asarray(Wo, np.float32).reshape(H, D, C)
    WoB[0, D, :] = np.asarray(bo, np.float32)
    common = {
        "hT": hT,
        "KbgT": KbgT,
        "Vbg": np.ascontiguousarray(np.asarray(V_bg, np.float32)),
        "Wq": np.asarray(Wq, np.float32),
        "Wk": np.asarray(Wk, np.float32),
        "Wv": np.asarray(Wv, np.float32),
        "WoB": WoB,
    }
    qs = L // N_CORES
    return [
        dict(common, hqT=np.ascontiguousarray(hT[:, qs * c : qs * (c + 1)]))
        for c in range(N_CORES)
    ]


_NC_CACHE = {}


def kernel(hidden_states, K_bg, V_bg, Wq, Wk, Wv, Wo, bo):
    if "nc" not in _NC_CACHE:
        _NC_CACHE["nc"] = build_bass()
    nc = _NC_CACHE["nc"]
    in_maps = make_in_maps(hidden_states, K_bg, V_bg, Wq, Wk, Wv, Wo, bo)
    from concourse import bass2jax

    results = bass2jax.run_bass_via_pjrt(nc, in_maps, n_cores=N_CORES)
    out = np.concatenate([results[c]["out"] for c in range(N_CORES)], axis=0)
    return out.reshape(B, L, C)


# revision 12
# speedup vs baseline: 1.0824x; 1.0824x over previous
"""CARC attention processor kernel for 8 Trainium2 NeuronCores.

Reference computation (B=1, L=4096, C=640, H=10, D=64):
    q/k/v = hidden @ Wq/Wk/Wv, split into 10 heads of 64
    k_cat = [k, 0.42*K_bg], v_cat = [v, 0.42*V_bg]   (key length 8192)
    out   = softmax(q k_cat^T / 8) v_cat, heads merged, @ Wo + bo

Sharding: queries are split 512 per core; every core computes all 10 heads
for its queries (k/v projections replicated per core — cheap relative to
attention).  Output is a disjoint row-slice per core; the host concatenates.

All matmuls run in bf16 with fp32 PSUM accumulation.  Softmax skips the
max-subtraction (scores are ~N(0,1); exp runs on ScalarE with the 1/8 scale
folded in, and the 0.42 key-side scale folded into the bg exp scale).  The
softmax denominator comes from a ones-column appended to V in the probs@V
matmul; the output-projection bias is folded in as a 65th row of Wo against
the ctx ones-column.

Heads are processed in pairs: projections compute both heads of a pair in
one matmul stream (head A on partitions 0-63, head B on 64-127), exp reads
1024-wide (two PSUM banks) per instruction, and the A/B score tiles
alternate through a shared 2-slot PSUM pool so ScalarE (the critical
engine) never starves.
"""

import numpy as np

import concourse.bass as bass
import concourse.mybir as mybir
import concourse.tile as tile
from concourse.masks import make_identity

F32 = mybir.dt.float32
BF16 = mybir.dt.bfloat16
AF = mybir.ActivationFunctionType

# Problem constants (hardcoded per contract)
B, L, C = 1, 4096, 640
H, D = 10, 64
ALPHA = 0.42
N_CORES = 8
SCALE = 1.0 / np.sqrt(D)  # 0.125


class Cfg:
    def __init__(self, H=H, C=C, Lk=L, Q=L // N_CORES):
        assert C % 128 == 0 and Lk % 1024 == 0 and Q % 128 == 0 and Q <= 512
        assert H % 2 == 0
        self.H, self.C, self.Lk, self.Q = H, C, Lk, Q
        self.n_cc = C // 128      # contraction chunks for projections
        self.n_kt = Lk // 128     # key tiles per source (self / bg)
        self.n_qt = Q // 128      # query tiles of this core


def emit(nc: bass.Bass, cfg: Cfg):
    Hh, Cc, Lk, Q = cfg.H, cfg.C, cfg.Lk, cfg.Q
    n_cc, n_kt, n_qt = cfg.n_cc, cfg.n_kt, cfg.n_qt
    n_pair = Hh // 2

    hT = nc.declare_dram_parameter("hT", [Cc, Lk], F32, isOutput=False)
    hqT = nc.declare_dram_parameter("hqT", [Cc, Q], F32, isOutput=False)
    kbgT = nc.declare_dram_parameter("KbgT", [Hh, D, Lk], F32, isOutput=False)
    vbg = nc.declare_dram_parameter("Vbg", [Hh, Lk, D], F32, isOutput=False)
    wq = nc.declare_dram_parameter("Wq", [Cc, Cc], F32, isOutput=False)
    wk = nc.declare_dram_parameter("Wk", [Cc, Cc], F32, isOutput=False)
    wv = nc.declare_dram_parameter("Wv", [Cc, Cc], F32, isOutput=False)
    wob = nc.declare_dram_parameter("WoB", [Hh, D + 1, Cc], F32, isOutput=False)
    out = nc.declare_dram_parameter("out", [Q, Cc], F32, isOutput=True)

    with tile.TileContext(nc) as tc:
        with (
            tc.tile_pool(name="singles", bufs=1) as singles,
            tc.tile_pool(name="stage", bufs=1) as stage,
            tc.tile_pool(name="bgstage", bufs=2) as bgstage,
            tc.tile_pool(name="kv", bufs=2) as kv,
            tc.tile_pool(name="probs", bufs=3) as probs_pool,
            tc.tile_pool(name="outsb", bufs=2) as outsb_pool,
            tc.tile_pool(name="ps_a", bufs=2, space="PSUM") as ps_a,
            tc.tile_pool(name="ps_sc", bufs=2, space="PSUM") as ps_sc,
            tc.tile_pool(name="ps_ctx", bufs=1, space="PSUM") as ps_ctx,
        ):
            # ---- persistent SBUF tensors ----
            hT_bf = singles.tile([128, n_cc, Lk], BF16, tag="hT_bf")
            hq_bf = singles.tile([128, n_cc, Q], BF16, tag="hq_bf")
            wq_bf = singles.tile([128, n_cc, Cc], BF16, tag="wq_bf")
            wk_bf = singles.tile([128, n_cc, Cc], BF16, tag="wk_bf")
            wv_bf = singles.tile([128, n_cc, Cc], BF16, tag="wv_bf")
            wob_bf = singles.tile([D + 1, Hh, Cc], BF16, tag="wob_bf")
            qT2_all = singles.tile([128, n_pair, Q], BF16, tag="qT2_all")
            ctx_all = singles.tile([128, Hh * n_qt, D + 1], BF16, tag="ctx_all")
            ctxT_all = singles.tile([D + 1, Hh * n_qt, 128], BF16, tag="ctxT_all")
            ident = singles.tile([128, 128], BF16, tag="ident")
            make_identity(nc, ident)

            # ---- load + cast hidden (transposed) and weights ----
            for i in range(n_cc):
                st = stage.tile([128, Lk], F32, tag="stage")
                nc.sync.dma_start(out=st, in_=hT[128 * i : 128 * (i + 1), :])
                nc.gpsimd.tensor_copy(out=hT_bf[:, i, :], in_=st)
            for i in range(n_cc):
                st = stage.tile([128, Q], F32, tag="stage")
                nc.sync.dma_start(out=st, in_=hqT[128 * i : 128 * (i + 1), :])
                nc.vector.tensor_copy(out=hq_bf[:, i, :], in_=st)
            for w_dram, w_sb in ((wq, wq_bf), (wk, wk_bf), (wv, wv_bf)):
                st = stage.tile([128, n_cc, Cc], F32, tag="stage")
                nc.sync.dma_start(
                    out=st, in_=w_dram.rearrange("(i p) n -> p i n", p=128)
                )
                nc.vector.tensor_copy(out=w_sb, in_=st)
            hh = Hh // 2
            for half in range(2):
                st = stage.tile([D + 1, hh, Cc], F32, tag="stage")
                nc.sync.dma_start(
                    out=st,
                    in_=wob[half * hh : (half + 1) * hh].rearrange("h p n -> p h n"),
                )
                nc.vector.tensor_copy(
                    out=wob_bf[:, half * hh : (half + 1) * hh, :], in_=st
                )

            # ---- q projections, head pairs packed on partitions ----
            for p in range(n_pair):
                ps = ps_a.tile([128, Q], F32, tag="ps_a", name=f"qps{p}")
                for i in range(n_cc):
                    nc.tensor.matmul(
                        ps,
                        lhsT=wq_bf[:, i, 128 * p : 128 * (p + 1)],
                        rhs=hq_bf[:, i, :],
                        start=(i == 0),
                        stop=(i == n_cc - 1),
                    )
                nc.vector.tensor_copy(out=qT2_all[:, p, :], in_=ps)

            # ---- per head-pair: project k/v, load bg kv, attention ----
            for p in range(n_pair):
                kT2 = kv.tile([128, Lk], BF16, tag="kT")
                v2t = kv.tile([128, n_kt, 2 * (D + 1)], BF16, tag="v")
                kbg2 = kv.tile([128, Lk], BF16, tag="kbg")
                vbg2 = kv.tile([128, n_kt, 2 * (D + 1)], BF16, tag="vbg")

                # kT2 = (hidden @ Wk_pair)^T, head A on partitions 0-63
                for t in range(Lk // 512):
                    ps = ps_a.tile([128, 512], F32, tag="ps_a", name=f"kps{p}{t}")
                    for i in range(n_cc):
                        nc.tensor.matmul(
                            ps,
                            lhsT=wk_bf[:, i, 128 * p : 128 * (p + 1)],
                            rhs=hT_bf[:, i, 512 * t : 512 * (t + 1)],
                            start=(i == 0),
                            stop=(i == n_cc - 1),
                        )
                    nc.vector.tensor_copy(
                        out=kT2[:, 512 * t : 512 * (t + 1)], in_=ps
                    )
                # v natural [keys, D] for both heads (+ones cols)
                for kt in range(n_kt):
                    ps = ps_a.tile([128, 128], F32, tag="ps_a", name=f"vps{p}{kt}")
                    for i in range(n_cc):
                        nc.tensor.matmul(
                            ps,
                            lhsT=hT_bf[:, i, 128 * kt : 128 * (kt + 1)],
                            rhs=wv_bf[:, i, 128 * p : 128 * (p + 1)],
                            start=(i == 0),
                            stop=(i == n_cc - 1),
                        )
                    nc.vector.tensor_copy(out=v2t[:, kt, 0:D], in_=ps[:, 0:D])
                    nc.vector.tensor_copy(
                        out=v2t[:, kt, D + 1 : 2 * D + 1], in_=ps[:, D : 2 * D]
                    )
                nc.vector.memset(v2t[:, :, D : D + 1], 1.0)
                nc.vector.memset(v2t[:, :, 2 * D + 1 : 2 * D + 2], 1.0)

                # bg K (transposed) and bg V (scaled by ALPHA at load),
                # staged in 1/4 pieces to bound SBUF staging space
                for p4 in range(4):
                    lw = Lk // 4
                    tw = n_kt // 4
                    st = bgstage.tile([128, lw], F32, tag="kbg_st", name=f"kst{p}{p4}")
                    nc.sync.dma_start(
                        out=st[0:D, :], in_=kbgT[2 * p, :, lw * p4 : lw * (p4 + 1)]
                    )
                    nc.sync.dma_start(
                        out=st[D : 2 * D, :],
                        in_=kbgT[2 * p + 1, :, lw * p4 : lw * (p4 + 1)],
                    )
                    nc.gpsimd.tensor_copy(
                        out=kbg2[:, lw * p4 : lw * (p4 + 1)], in_=st
                    )
                    st2 = bgstage.tile(
                        [128, tw, 2 * D], F32, tag="vbg_st", name=f"vst{p}{p4}"
                    )
                    for hi in range(2):
                        nc.sync.dma_start(
                            out=st2[:, :, D * hi : D * (hi + 1)],
                            in_=vbg[
                                2 * p + hi, lw * p4 : lw * (p4 + 1), :
                            ].rearrange("(kt q) d -> q kt d", q=128),
                        )
                        nc.gpsimd.tensor_scalar_mul(
                            vbg2[
                                :,
                                tw * p4 : tw * (p4 + 1),
                                (D + 1) * hi : (D + 1) * hi + D,
                            ],
                            st2[:, :, D * hi : D * (hi + 1)],
                            ALPHA,
                        )
                nc.vector.memset(vbg2[:, :, D : D + 1], 1.0)
                nc.vector.memset(vbg2[:, :, 2 * D + 1 : 2 * D + 2], 1.0)

                # ---- attention for the pair ----
                # ctx accumulators: head A in PSUM bank 0, head B in bank 1
                ctx2 = ps_ctx.tile([128, 2, 512], F32, tag="ctx", name=f"ctx{p}")
                n_k2 = n_kt // 2
                for src in range(2):  # 0=self keys, 1=bg keys
                    kk = kT2 if src == 0 else kbg2
                    vv = v2t if src == 0 else vbg2
                    e_scale = SCALE if src == 0 else SCALE * ALPHA
                    for k2 in range(n_k2):
                        first = src == 0 and k2 == 0
                        last = src == 1 and k2 == n_k2 - 1
                        for hi in range(2):
                            sc = ps_sc.tile(
                                [128, 2, Q], F32, tag="sc", name=f"sc{p}{src}{k2}{hi}"
                            )
                            for j in range(2):
                                kt = 2 * k2 + j
                                nc.tensor.matmul(
                                    sc[:, j, :],
                                    lhsT=kk[
                                        D * hi : D * (hi + 1),
                                        128 * kt : 128 * (kt + 1),
                                    ],
                                    rhs=qT2_all[D * hi : D * (hi + 1), p, :],
                                    start=True,
                                    stop=True,
                                    tile_position=(D * hi, 0),
                                )
                            pr = probs_pool.tile(
                                [128, 2, Q], BF16, tag="pr", name=f"pr{p}{src}{k2}{hi}"
                            )
                            nc.scalar.activation(pr, sc, AF.Exp, scale=e_scale)
                            for j in range(2):
                                kt = 2 * k2 + j
                                for qt in range(n_qt):
                                    # One accumulation group per PSUM bank:
                                    # start marks the whole 2KB bank
                                    # pending-zero, so only the first matmul
                                    # of the bank starts, and each qt
                                    # sub-region is lazily zeroed on its
                                    # first write.
                                    nc.tensor.matmul(
                                        ctx2[
                                            :, hi, 65 * qt : 65 * qt + D + 1
                                        ],
                                        lhsT=pr[:, j, 128 * qt : 128 * (qt + 1)],
                                        rhs=vv[
                                            :, kt, (D + 1) * hi : (D + 1) * (hi + 1)
                                        ],
                                        start=(first and j == 0 and qt == 0),
                                        stop=(last and j == 1 and qt == n_qt - 1),
                                    )
                # normalize + transpose ctx for the output projection
                for hi in range(2):
                    h = 2 * p + hi
                    for qt in range(n_qt):
                        jj = h * n_qt + qt
                        r = probs_pool.tile(
                            [128, 1], F32, tag="recip", name=f"r{h}{qt}"
                        )
                        nc.vector.reciprocal(
                            r, ctx2[:, hi, 65 * qt + D : 65 * qt + D + 1]
                        )
                        nc.vector.tensor_scalar_mul(
                            ctx_all[:, jj, 0:D],
                            ctx2[:, hi, 65 * qt : 65 * qt + D],
                            r,
                        )
                        nc.vector.memset(ctx_all[:, jj, D : D + 1], 1.0)
                        tr = ps_a.tile(
                            [D + 1, 128], BF16, tag="ps_a", name=f"tr{h}{qt}"
                        )
                        nc.tensor.transpose(tr, ctx_all[:, jj, :], ident)
                        nc.vector.tensor_copy(out=ctxT_all[:, jj, :], in_=tr)

            # ---- output projection: out[qt] = sum_h ctxT_h^T @ WoB_h ----
            for qt in range(n_qt):
                o_sb = outsb_pool.tile([128, Cc], F32, tag="o_sb")
                for n0 in range(0, Cc, 512):
                    nw = min(512, Cc - n0)
                    ps = ps_sc.tile([128, 2, Q], F32, tag="sc", name=f"ops{qt}{n0}")
                    for h in range(Hh):
                        nc.tensor.matmul(
                            ps[:, 0, 0:nw],
                            lhsT=ctxT_all[:, h * n_qt + qt, :],
                            rhs=wob_bf[:, h, n0 : n0 + nw],
                            start=(h == 0),
                            stop=(h == Hh - 1),
                        )
                    nc.vector.tensor_copy(out=o_sb[:, n0 : n0 + nw], in_=ps[:, 0, 0:nw])
                nc.sync.dma_start(
                    out=out[128 * qt : 128 * (qt + 1), :], in_=o_sb
                )
    return nc


def split_waits(nc, limit=1):
    """This container's walrus rejects >limit sync waits per instruction;
    hoist excess waits onto standalone EventSemaphore instructions."""
    cnt = 0
    for f in nc.m.functions:
        for bb in f.blocks:
            fixed = []
            for inst in bb.instructions:
                si = inst.sync_info
                if si is not None and len(si.on_wait) > limit:
                    waits = list(si.on_wait)
                    extra, keep = waits[:-limit], waits[-limit:]
                    for w in extra:
                        cnt += 1
                        ev = mybir.InstEventSemaphore(
                            name=f"I-waitsplit-{cnt}", ins=[], outs=[]
                        )
                        ev.engine = inst.engine
                        ev.sync_info = mybir.SyncInfo(on_wait=[w], on_update=[])
                        nc.register_instruction(ev)
                        fixed.append(ev)
                    si.on_wait = keep
                fixed.append(inst)
            bb.instructions[:] = fixed
    return cnt


def build_bass(cfg: Cfg | None = None):
    cfg = cfg or Cfg()
    nc = bass.Bass()
    emit(nc, cfg)
    split_waits(nc)
    return nc


def make_in_maps(hidden_states, K_bg, V_bg, Wq, Wk, Wv, Wo, bo):
    hT = np.ascontiguousarray(np.asarray(hidden_states, np.float32)[0].T)
    KbgT = np.ascontiguousarray(np.asarray(K_bg, np.float32).transpose(0, 2, 1))
    WoB = np.zeros((H, D + 1, C), np.float32)
    WoB[:, :D, :] = np.asarray(Wo, np.float32).reshape(H, D, C)
    WoB[0, D, :] = np.asarray(bo, np.float32)
    common = {
        "hT": hT,
        "KbgT": KbgT,
        "Vbg": np.ascontiguousarray(np.asarray(V_bg, np.float32)),
        "Wq": np.asarray(Wq, np.float32),
        "Wk": np.asarray(Wk, np.float32),
        "Wv": np.asarray(Wv, np.float32),
        "WoB": WoB,
    }
    qs = L // N_CORES
    return [
        dict(common, hqT=np.ascontiguousarray(hT[:, qs * c : qs * (c + 1)]))
        for c in range(N_CORES)
    ]


_NC_CACHE = {}


def kernel(hidden_states, K_bg, V_bg, Wq, Wk, Wv, Wo, bo):
    if "nc" not in _NC_CACHE:
        _NC_CACHE["nc"] = build_bass()
    nc = _NC_CACHE["nc"]
    in_maps = make_in_maps(hidden_states, K_bg, V_bg, Wq, Wk, Wv, Wo, bo)
    from concourse import bass2jax

    results = bass2jax.run_bass_via_pjrt(nc, in_maps, n_cores=N_CORES)
    out = np.concatenate([results[c]["out"] for c in range(N_CORES)], axis=0)
    return out.reshape(B, L, C)


# revision 16
# speedup vs baseline: 1.3546x; 1.2515x over previous
"""CARC attention processor kernel for 8 Trainium2 NeuronCores.

Reference computation (B=1, L=4096, C=640, H=10, D=64):
    q/k/v = hidden @ Wq/Wk/Wv, split into 10 heads of 64
    k_cat = [k, 0.42*K_bg], v_cat = [v, 0.42*V_bg]   (key length 8192)
    out   = softmax(q k_cat^T / 8) v_cat, heads merged, @ Wo + bo

Sharding: queries are split 512 per core; every core computes all 10 heads
for its queries (k/v projections replicated per core — cheap relative to
attention).  Output is a disjoint row-slice per core; the host concatenates.

All matmuls run in bf16 with fp32 PSUM accumulation.  Softmax skips the
max-subtraction (scores are ~N(0,1); exp runs on ScalarE with the 1/8 scale
folded in, and the 0.42 key-side scale folded into the bg exp scale).  The
softmax denominator comes from a ones-column appended to V in the probs@V
matmul; the output-projection bias is folded in as a 65th row of Wo against
the ctx ones-column.

Heads are processed in pairs: projections compute both heads of a pair in
one matmul stream (head A on partitions 0-63, head B on 64-127), exp reads
1024-wide (two PSUM banks) per instruction, and the A/B score tiles
alternate through a shared 2-slot PSUM pool so ScalarE (the critical
engine) never starves.
"""

import numpy as np

import concourse.bass as bass
import concourse.mybir as mybir
import concourse.tile as tile
from concourse.masks import make_identity

F32 = mybir.dt.float32
BF16 = mybir.dt.bfloat16
AF = mybir.ActivationFunctionType

# Problem constants (hardcoded per contract)
B, L, C = 1, 4096, 640
H, D = 10, 64
ALPHA = 0.42
N_CORES = 8
SCALE = 1.0 / np.sqrt(D)  # 0.125


class Cfg:
    def __init__(self, H=H, C=C, Lk=L, Q=L // N_CORES):
        assert C % 128 == 0 and Lk % 1024 == 0 and Q % 128 == 0 and Q <= 512
        assert H % 2 == 0
        self.H, self.C, self.Lk, self.Q = H, C, Lk, Q
        self.n_cc = C // 128      # contraction chunks for projections
        self.n_kt = Lk // 128     # key tiles per source (self / bg)
        self.n_qt = Q // 128      # query tiles of this core


def emit(nc: bass.Bass, cfg: Cfg):
    Hh, Cc, Lk, Q = cfg.H, cfg.C, cfg.Lk, cfg.Q
    n_cc, n_kt, n_qt = cfg.n_cc, cfg.n_kt, cfg.n_qt
    n_pair = Hh // 2

    hT = nc.declare_dram_parameter("hT", [Cc, Lk], F32, isOutput=False)
    hqT = nc.declare_dram_parameter("hqT", [Cc, Q], F32, isOutput=False)
    kbgT = nc.declare_dram_parameter("KbgT", [Hh, D, Lk], F32, isOutput=False)
    vbg = nc.declare_dram_parameter("Vbg", [Hh, Lk, D], F32, isOutput=False)
    wq = nc.declare_dram_parameter("Wq", [Cc, Cc], F32, isOutput=False)
    wk = nc.declare_dram_parameter("Wk", [Cc, Cc], F32, isOutput=False)
    wv = nc.declare_dram_parameter("Wv", [Cc, Cc], F32, isOutput=False)
    wob = nc.declare_dram_parameter("WoB", [Hh, D + 1, Cc], F32, isOutput=False)
    out = nc.declare_dram_parameter("out", [Q, Cc], F32, isOutput=True)

    with tile.TileContext(nc) as tc:
        with (
            tc.tile_pool(name="singles", bufs=1) as singles,
            tc.tile_pool(name="stage", bufs=1) as stage,
            tc.tile_pool(name="bgstage", bufs=2) as bgstage,
            tc.tile_pool(name="kv", bufs=2) as kv,
            tc.tile_pool(name="probs", bufs=3) as probs_pool,
            tc.tile_pool(name="outsb", bufs=2) as outsb_pool,
            tc.tile_pool(name="ps_a", bufs=2, space="PSUM") as ps_a,
            tc.tile_pool(name="ps_sc", bufs=2, space="PSUM") as ps_sc,
            tc.tile_pool(name="ps_ctx", bufs=1, space="PSUM") as ps_ctx,
        ):
            # ---- persistent SBUF tensors ----
            hT_bf = singles.tile([128, n_cc, Lk], BF16, tag="hT_bf")
            hq_bf = singles.tile([128, n_cc, Q], BF16, tag="hq_bf")
            wq_bf = singles.tile([128, n_cc, Cc], BF16, tag="wq_bf")
            wk_bf = singles.tile([128, n_cc, Cc], BF16, tag="wk_bf")
            wv_bf = singles.tile([128, n_cc, Cc], BF16, tag="wv_bf")
            wob_bf = singles.tile([D + 1, Hh, Cc], BF16, tag="wob_bf")
            qT2_all = singles.tile([128, n_pair, Q], BF16, tag="qT2_all")
            ctx_all = singles.tile([128, Hh * n_qt, D + 1], BF16, tag="ctx_all")
            ctxT_all = singles.tile([D + 1, Hh * n_qt, 128], BF16, tag="ctxT_all")
            ident = singles.tile([128, 128], BF16, tag="ident")
            make_identity(nc, ident)

            # ---- load + cast hidden (transposed) and weights ----
            for i in range(n_cc):
                st = stage.tile([128, Lk], F32, tag="stage")
                nc.sync.dma_start(out=st, in_=hT[128 * i : 128 * (i + 1), :])
                nc.vector.tensor_copy(out=hT_bf[:, i, :], in_=st)
            for i in range(n_cc):
                st = stage.tile([128, Q], F32, tag="stage")
                nc.sync.dma_start(out=st, in_=hqT[128 * i : 128 * (i + 1), :])
                nc.vector.tensor_copy(out=hq_bf[:, i, :], in_=st)
            for w_dram, w_sb in ((wq, wq_bf), (wk, wk_bf), (wv, wv_bf)):
                st = stage.tile([128, n_cc, Cc], F32, tag="stage")
                nc.sync.dma_start(
                    out=st, in_=w_dram.rearrange("(i p) n -> p i n", p=128)
                )
                nc.vector.tensor_copy(out=w_sb, in_=st)
            hh = Hh // 2
            for half in range(2):
                st = stage.tile([D + 1, hh, Cc], F32, tag="stage")
                nc.sync.dma_start(
                    out=st,
                    in_=wob[half * hh : (half + 1) * hh].rearrange("h p n -> p h n"),
                )
                nc.vector.tensor_copy(
                    out=wob_bf[:, half * hh : (half + 1) * hh, :], in_=st
                )

            # ---- q projections, head pairs packed on partitions ----
            for p in range(n_pair):
                ps = ps_a.tile([128, Q], F32, tag="ps_a", name=f"qps{p}")
                for i in range(n_cc):
                    nc.tensor.matmul(
                        ps,
                        lhsT=wq_bf[:, i, 128 * p : 128 * (p + 1)],
                        rhs=hq_bf[:, i, :],
                        start=(i == 0),
                        stop=(i == n_cc - 1),
                    )
                nc.vector.tensor_copy(out=qT2_all[:, p, :], in_=ps)

            # ---- per head-pair: project k/v, load bg kv, attention ----
            for p in range(n_pair):
                kT2 = kv.tile([128, Lk], BF16, tag="kT")
                v2t = kv.tile([128, n_kt, 2 * (D + 1)], BF16, tag="v")
                kbg2 = kv.tile([128, Lk], BF16, tag="kbg")
                vbg2 = kv.tile([128, n_kt, 2 * (D + 1)], BF16, tag="vbg")

                # kT2 = (hidden @ Wk_pair)^T, head A on partitions 0-63
                for t in range(Lk // 512):
                    ps = ps_a.tile([128, 512], F32, tag="ps_a", name=f"kps{p}{t}")
                    for i in range(n_cc):
                        nc.tensor.matmul(
                            ps,
                            lhsT=wk_bf[:, i, 128 * p : 128 * (p + 1)],
                            rhs=hT_bf[:, i, 512 * t : 512 * (t + 1)],
                            start=(i == 0),
                            stop=(i == n_cc - 1),
                        )
                    nc.vector.tensor_copy(
                        out=kT2[:, 512 * t : 512 * (t + 1)], in_=ps
                    )
                # v natural [keys, D] for both heads (+ones cols)
                for kt in range(n_kt):
                    ps = ps_a.tile([128, 128], F32, tag="ps_a", name=f"vps{p}{kt}")
                    for i in range(n_cc):
                        nc.tensor.matmul(
                            ps,
                            lhsT=hT_bf[:, i, 128 * kt : 128 * (kt + 1)],
                            rhs=wv_bf[:, i, 128 * p : 128 * (p + 1)],
                            start=(i == 0),
                            stop=(i == n_cc - 1),
                        )
                    nc.vector.tensor_copy(out=v2t[:, kt, 0:D], in_=ps[:, 0:D])
                    nc.vector.tensor_copy(
                        out=v2t[:, kt, D + 1 : 2 * D + 1], in_=ps[:, D : 2 * D]
                    )
                nc.vector.memset(v2t[:, :, D : D + 1], 1.0)
                nc.vector.memset(v2t[:, :, 2 * D + 1 : 2 * D + 2], 1.0)

                # bg K (transposed) and bg V (scaled by ALPHA at load),
                # staged in 1/4 pieces to bound SBUF staging space
                for p4 in range(4):
                    lw = Lk // 4
                    tw = n_kt // 4
                    st = bgstage.tile([128, lw], F32, tag="kbg_st", name=f"kst{p}{p4}")
                    nc.sync.dma_start(
                        out=st[0:D, :], in_=kbgT[2 * p, :, lw * p4 : lw * (p4 + 1)]
                    )
                    nc.sync.dma_start(
                        out=st[D : 2 * D, :],
                        in_=kbgT[2 * p + 1, :, lw * p4 : lw * (p4 + 1)],
                    )
                    nc.vector.tensor_copy(
                        out=kbg2[:, lw * p4 : lw * (p4 + 1)], in_=st
                    )
                    st2 = bgstage.tile(
                        [128, tw, 2 * D], F32, tag="vbg_st", name=f"vst{p}{p4}"
                    )
                    for hi in range(2):
                        nc.sync.dma_start(
                            out=st2[:, :, D * hi : D * (hi + 1)],
                            in_=vbg[
                                2 * p + hi, lw * p4 : lw * (p4 + 1), :
                            ].rearrange("(kt q) d -> q kt d", q=128),
                        )
                        nc.vector.tensor_scalar_mul(
                            vbg2[
                                :,
                                tw * p4 : tw * (p4 + 1),
                                (D + 1) * hi : (D + 1) * hi + D,
                            ],
                            st2[:, :, D * hi : D * (hi + 1)],
                            ALPHA,
                        )
                nc.vector.memset(vbg2[:, :, D : D + 1], 1.0)
                nc.vector.memset(vbg2[:, :, 2 * D + 1 : 2 * D + 2], 1.0)

                # ---- attention for the pair ----
                # ctx accumulators: head A in PSUM bank 0, head B in bank 1
                ctx2 = ps_ctx.tile([128, 2, 512], F32, tag="ctx", name=f"ctx{p}")
                n_k2 = n_kt // 2
                for src in range(2):  # 0=self keys, 1=bg keys
                    kk = kT2 if src == 0 else kbg2
                    vv = v2t if src == 0 else vbg2
                    e_scale = SCALE if src == 0 else SCALE * ALPHA
                    for k2 in range(n_k2):
                        first = src == 0 and k2 == 0
                        last = src == 1 and k2 == n_k2 - 1
                        # QK for heads A/B issued back-to-back per key tile:
                        # distinct PE row groups (tile_position) let the two
                        # K=64 matmuls stream concurrently.
                        scs = []
                        for hi in range(2):
                            scs.append(
                                ps_sc.tile(
                                    [128, 2, Q],
                                    F32,
                                    tag="sc",
                                    name=f"sc{p}{src}{k2}{hi}",
                                )
                            )
                        for j in range(2):
                            kt = 2 * k2 + j
                            for hi in range(2):
                                nc.tensor.matmul(
                                    scs[hi][:, j, :],
                                    lhsT=kk[
                                        D * hi : D * (hi + 1),
                                        128 * kt : 128 * (kt + 1),
                                    ],
                                    rhs=qT2_all[D * hi : D * (hi + 1), p, :],
                                    start=True,
                                    stop=True,
                                    tile_position=(D * hi, 0),
                                )
                        prs = []
                        for hi in range(2):
                            pr = probs_pool.tile(
                                [128, 2, Q], BF16, tag="pr", name=f"pr{p}{src}{k2}{hi}"
                            )
                            nc.scalar.activation(pr, scs[hi], AF.Exp, scale=e_scale)
                            prs.append(pr)
                        for hi in range(2):
                            for j in range(2):
                                kt = 2 * k2 + j
                                for qt in range(n_qt):
                                    # One accumulation group per PSUM bank:
                                    # start marks the whole 2KB bank
                                    # pending-zero, so only the first matmul
                                    # of the bank starts, and each qt
                                    # sub-region is lazily zeroed on its
                                    # first write.
                                    nc.tensor.matmul(
                                        ctx2[
                                            :, hi, 65 * qt : 65 * qt + D + 1
                                        ],
                                        lhsT=prs[hi][
                                            :, j, 128 * qt : 128 * (qt + 1)
                                        ],
                                        rhs=vv[
                                            :, kt, (D + 1) * hi : (D + 1) * (hi + 1)
                                        ],
                                        start=(first and j == 0 and qt == 0),
                                        stop=(last and j == 1 and qt == n_qt - 1),
                                    )
                # normalize + transpose ctx for the output projection
                for hi in range(2):
                    h = 2 * p + hi
                    for qt in range(n_qt):
                        jj = h * n_qt + qt
                        r = probs_pool.tile(
                            [128, 1], F32, tag="recip", name=f"r{h}{qt}"
                        )
                        nc.vector.reciprocal(
                            r, ctx2[:, hi, 65 * qt + D : 65 * qt + D + 1]
                        )
                        nc.vector.tensor_scalar_mul(
                            ctx_all[:, jj, 0:D],
                            ctx2[:, hi, 65 * qt : 65 * qt + D],
                            r,
                        )
                        nc.vector.memset(ctx_all[:, jj, D : D + 1], 1.0)
                        tr = ps_a.tile(
                            [D + 1, 128], BF16, tag="ps_a", name=f"tr{h}{qt}"
                        )
                        nc.tensor.transpose(tr, ctx_all[:, jj, :], ident)
                        nc.vector.tensor_copy(out=ctxT_all[:, jj, :], in_=tr)

            # ---- output projection: out[qt] = sum_h ctxT_h^T @ WoB_h ----
            for qt in range(n_qt):
                o_sb = outsb_pool.tile([128, Cc], F32, tag="o_sb")
                for n0 in range(0, Cc, 512):
                    nw = min(512, Cc - n0)
                    ps = ps_sc.tile([128, 2, Q], F32, tag="sc", name=f"ops{qt}{n0}")
                    for h in range(Hh):
                        nc.tensor.matmul(
                            ps[:, 0, 0:nw],
                            lhsT=ctxT_all[:, h * n_qt + qt, :],
                            rhs=wob_bf[:, h, n0 : n0 + nw],
                            start=(h == 0),
                            stop=(h == Hh - 1),
                        )
                    nc.vector.tensor_copy(out=o_sb[:, n0 : n0 + nw], in_=ps[:, 0, 0:nw])
                nc.sync.dma_start(
                    out=out[128 * qt : 128 * (qt + 1), :], in_=o_sb
                )
    return nc


def split_waits(nc, limit=1):
    """This container's walrus rejects >limit sync waits per instruction;
    hoist excess waits onto standalone EventSemaphore instructions."""
    cnt = 0
    for f in nc.m.functions:
        for bb in f.blocks:
            fixed = []
            for inst in bb.instructions:
                si = inst.sync_info
                if si is not None and len(si.on_wait) > limit:
                    waits = list(si.on_wait)
                    extra, keep = waits[:-limit], waits[-limit:]
                    for w in extra:
                        cnt += 1
                        ev = mybir.InstEventSemaphore(
                            name=f"I-waitsplit-{cnt}", ins=[], outs=[]
                        )
                        ev.engine = inst.engine
                        ev.sync_info = mybir.SyncInfo(on_wait=[w], on_update=[])
                        nc.register_instruction(ev)
                        fixed.append(ev)
                    si.on_wait = keep
                fixed.append(inst)
            bb.instructions[:] = fixed
    return cnt


def build_bass(cfg: Cfg | None = None):
    cfg = cfg or Cfg()
    nc = bass.Bass()
    emit(nc, cfg)
    split_waits(nc)
    return nc


def make_in_maps(hidden_states, K_bg, V_bg, Wq, Wk, Wv, Wo, bo):
    hT = np.ascontiguousarray(np.asarray(hidden_states, np.float32)[0].T)
    KbgT = np.ascontiguousarray(np.asarray(K_bg, np.float32).transpose(0, 2, 1))
    WoB = np.zeros((H, D + 1, C), np.float32)
    WoB[:, :D, :] = np.asarray(Wo, np.float32).reshape(H, D, C)
    WoB[0, D, :] = np.asarray(bo, np.float32)
    common = {
        "hT": hT,
        "KbgT": KbgT,
        "Vbg": np.ascontiguousarray(np.asarray(V_bg, np.float32)),
        "Wq": np.asarray(Wq, np.float32),
        "Wk": np.asarray(Wk, np.float32),
        "Wv": np.asarray(Wv, np.float32),
        "WoB": WoB,
    }
    qs = L // N_CORES
    return [
        dict(common, hqT=np.ascontiguousarray(hT[:, qs * c : qs * (c + 1)]))
        for c in range(N_CORES)
    ]


_NC_CACHE = {}


def kernel(hidden_states, K_bg, V_bg, Wq, Wk, Wv, Wo, bo):
    if "nc" not in _NC_CACHE:
        _NC_CACHE["nc"] = build_bass()
    nc = _NC_CACHE["nc"]
    in_maps = make_in_maps(hidden_states, K_bg, V_bg, Wq, Wk, Wv, Wo, bo)
    from concourse import bass2jax

    results = bass2jax.run_bass_via_pjrt(nc, in_maps, n_cores=N_CORES)
    out = np.concatenate([results[c]["out"] for c in range(N_CORES)], axis=0)
    return out.reshape(B, L, C)
